# revision 5
# baseline (speedup 1.0000x reference)
"""Trainium2 Bass kernel for the spiking net (IF neurons, T=28) firing-rate model.

Reference computation (per batch element b):
  xs[t, i] = x[b, 0, i, t]
  h1 = xs @ W1.T + b1            # [28, 32]
  s1 = IF-scan over t (v += h; spike = v>=1; hard reset v=0)
  h2 = s1 @ W2.T + b2            # [28, 10]
  s2 = IF-scan over t
  out[b] = s2.mean(axis=0)       # [10]

Mapping: pure data parallel over 8 NeuronCores (8192 batch each).
Per core, batch is packed as (bs in 4) x (tile in 4) x (bo in 512):
  - L1 matmul: lhsT = block-diag W1 [112=(bs,i), 128=(bs,o)], rhs = x tile
    [112, 512] per timestep (fp32, exact), accumulating h1 into PSUM.
  - IF scan: one fused custom DVE op per step:
      v' = select(v + h + b1 < 1, v + h + b1, 0)
    spike s = (v' == 0) via tensor_scalar is_equal (fp16 {0,1}, exact).
  - L2 matmul: lhsT = block-diag W2 [128=(bs,o1), 40=(bs,o2)] split into two
    fp16 terms (hi + lo); rhs = spikes (fp16 {0,1}) so products are exact and
    the weight split is accurate to ~2^-22. Three batch subgroups stack the
    40-row outputs to 120 PSUM partitions to keep DVE ops wide.
  - L2 IF scan + spike-count accumulation (GPSIMD scalar_tensor_tensor).
  - out = count / 28, DMA'd per (tile, subgroup) to a [10, 8192] output.
"""

import numpy as np

import concourse.bass as bass
import concourse.tile as tile
from concourse import bacc, mybir
from concourse.bass_utils import run_bass_kernel_spmd

F32 = mybir.dt.float32
F16 = mybir.dt.float16

N_CORES = 8
BC = 8192          # batch per core
BS = 4             # batch subgroups on L1 partitions (4*28=112)
NT = 4             # tiles per core
NB = 512           # batch per (bs, tile)
T = 28
SUBS = (256, 256)        # L2 batch subgroups; psum partition bases 0 and 64
SUBB = (0, 64)           # matmul out base partition must be in {0, 32, 64}

_CACHE = {}


def _register_dve_ops():
    from concourse import dve_ops
    from concourse.dve_spec import Spec, Src0, Src1, C0, Zero, One, select, lower, eq
    from concourse.dve_spec import _has_src1 as has_src1
    from concourse.dve_uop import DveOpSpec

    def reg(name, spec):
        if name in dve_ops._SUB_OPCODE_FOR_NAME:
            return next(o for o in dve_ops.OPS if o.name == name)
        row = dve_ops._CUSTOM_DVE_ROW_BASE + len(dve_ops.OPS)
        assert row < 0x20
        dve_ops._SUB_OPCODE_FOR_NAME[name] = row
        shas = {}
        for ver in ("v3", "v4"):
            s = DveOpSpec(name=name, opcode=row, uops=lower(spec, ver=ver),
                          rd1_en=has_src1(spec))
            shas[ver] = s.sha(ver)
        op = dve_ops.DveOp(name, spec, subdim=False, uops_sha=shas)
        dve_ops.OPS.append(op)
        dve_ops.CUSTOM_DVE_SPECS[name] = spec
        return op

    u = Src0 + Src1 + C0
    if_spec = Spec(
        body=select(u < One, u, Zero),
        reference=lambda in0, in1, s0: np.where(
            in0 + in1 + s0 < 1.0, in0 + in1 + s0, 0.0).astype(np.float32),
    )
    if_step = reg("IF_STEP_ANT", if_spec)

    acc_spec = Spec(
        body=Src0 + eq(Src1, Zero),
        reference=lambda in0, in1: (in0 + (in1 == 0.0)).astype(np.float32),
    )
    acc_spike = reg("ACC_SPIKE_ANT", acc_spec)
    return if_step, acc_spike


def _build_nc():
    IF_STEP, ACC_SPIKE = _register_dve_ops()
    nc = bacc.Bacc("TRN2", target_bir_lowering=False, debug=False,
                   num_devices=N_CORES)

    xd = nc.dram_tensor("x", [112, NT * NB * T], F32, kind="ExternalInput").ap()
    w1d = nc.dram_tensor("w1blk", [112, 128], F32, kind="ExternalInput").ap()
    b1d = nc.dram_tensor("b1blk", [128, 1], F32, kind="ExternalInput").ap()
    w2hd = nc.dram_tensor("w2h", [128, 40], F16, kind="ExternalInput").ap()
    w2ld = nc.dram_tensor("w2l", [128, 40], F16, kind="ExternalInput").ap()
    b2d = nc.dram_tensor("b2blk", [104, 1], F32, kind="ExternalInput").ap()
    yd = nc.dram_tensor("y", [10, BC], F32, kind="ExternalOutput").ap()
    # y[o2, b] with b = bs*2048 + tile*512 + sub_off + bo
    y4 = yd.rearrange("o (bs r) -> bs o r", bs=BS)  # [4, 10, 2048]

    with tile.TileContext(nc) as tc:
        with tc.tile_pool(name="consts", bufs=1) as cp, \
             tc.tile_pool(name="xp", bufs=2) as xp, \
             tc.tile_pool(name="s1p", bufs=2) as s1p, \
             tc.tile_pool(name="vp", bufs=1) as vp, \
             tc.tile_pool(name="accp", bufs=2) as accp, \
             tc.tile_pool(name="ps1", bufs=4, space="PSUM") as ps1, \
             tc.tile_pool(name="ps2", bufs=2, space="PSUM") as ps2:

            w1t = cp.tile([112, 128], F32)
            nc.sync.dma_start(w1t[:], w1d[:])
            b1t = cp.tile([128, 1], F32)
            nc.sync.dma_start(b1t[:], b1d[:])
            w2ht = cp.tile([128, 40], F16)
            nc.sync.dma_start(w2ht[:], w2hd[:])
            w2lt = cp.tile([128, 40], F16)
            nc.sync.dma_start(w2lt[:], w2ld[:])
            b2t = cp.tile([104, 1], F32)
            nc.sync.dma_start(b2t[:], b2d[:])
            vz = cp.tile([128, NB], F32)
            nc.gpsimd.memset(vz[:], 0.0)

            for n in range(NT):
                # ---- load x tile [112, (bo t)] (contiguous per partition)
                xt = xp.tile([112, NB * T], F32)
                nc.sync.dma_start(xt[:], xd[:, n * NB * T:(n + 1) * NB * T])
                x3 = xt[:].rearrange("p (bo t) -> p bo t", t=T)

                s1 = s1p.tile([128, T * NB], F16)

                # ---- layer 1: matmul + IF scan + spike extract
                vcur = vp.tile([128, NB], F32, tag="v1a")
                vnxt = vp.tile([128, NB], F32, tag="v1b")
                for t in range(T):
                    ps = ps1.tile([128, NB], F32)
                    nc.tensor.matmul(ps[:], w1t[:], x3[:, :, t],
                                     start=True, stop=True)
                    vin = vz if t == 0 else vcur
                    nc.vector._custom_dve(IF_STEP, out=vnxt[:], in0=vin[:],
                                          in1=ps[:], s0=b1t[:])
                    nc.vector.tensor_scalar(
                        s1[:, t * NB:(t + 1) * NB], vnxt[:], 0.0, None,
                        mybir.AluOpType.is_equal)
                    vcur, vnxt = vnxt, vcur

                # ---- layer 2: matmul (2-term fp16) + IF scan + spike count
                acc = accp.tile([104, SUBS[0]], F32)
                nc.gpsimd.memset(acc[:], 0.0)
                v2cur = vp.tile([104, SUBS[0]], F32, tag="v2a")
                v2nxt = vp.tile([104, SUBS[0]], F32, tag="v2b")
                for t in range(T):
                    p2 = ps2.tile([104, SUBS[0]], F32)
                    off = 0
                    for si, sw in enumerate(SUBS):
                        rhs = s1[:, t * NB + off: t * NB + off + sw]
                        nc.tensor.matmul(p2[SUBB[si]:SUBB[si] + 40, :sw],
                                         w2ht[:], rhs, start=True, stop=False)
                        off += sw
                    off = 0
                    for si, sw in enumerate(SUBS):
                        rhs = s1[:, t * NB + off: t * NB + off + sw]
                        nc.tensor.matmul(p2[SUBB[si]:SUBB[si] + 40, :sw],
                                         w2lt[:], rhs, start=False, stop=True)
                        off += sw
                    v2in = vz[:104, :SUBS[0]] if t == 0 else v2cur[:]
                    nc.vector._custom_dve(IF_STEP, out=v2nxt[:], in0=v2in,
                                          in1=p2[:], s0=b2t[:])
                    nc.vector._custom_dve(ACC_SPIKE, out=acc[:],
                                          in0=acc[:], in1=v2nxt[:])
                    v2cur, v2nxt = v2nxt, v2cur

                # ---- finalize: out = acc / T, DMA per subgroup
                nc.vector.tensor_scalar(acc[:], acc[:], 1.0 / T, None,
                                        mybir.AluOpType.mult)
                off = 0
                for si, sw in enumerate(SUBS):
                    dst = y4[:, :, n * NB + off: n * NB + off + sw]
                    nc.sync.dma_start(dst, acc[SUBB[si]:SUBB[si] + 40, :sw])
                    off += sw
    nc.compile()
    return nc


def _host_prep(x, W1, b1, W2, b2):
    B = x.shape[0]
    bc = B // N_CORES
    X = np.ascontiguousarray(x.reshape(B, 28, 28), dtype=np.float32)
    # per core: [bs, tile, bo, i, t] -> [(bs i), (tile bo t)]
    Xc = X.reshape(N_CORES, BS, NT, NB, 28, 28).transpose(0, 1, 4, 2, 3, 5)
    Xc = np.ascontiguousarray(Xc).reshape(N_CORES, 112, NT * NB * T)

    w1blk = np.zeros((112, 128), np.float32)
    for bs in range(BS):
        w1blk[bs * 28:(bs + 1) * 28, bs * 32:(bs + 1) * 32] = W1.T
    b1blk = np.tile(b1, BS).reshape(128, 1).astype(np.float32)

    w2blk = np.zeros((128, 40), np.float32)
    for bs in range(BS):
        w2blk[bs * 32:(bs + 1) * 32, bs * 10:(bs + 1) * 10] = W2.T
    w2h = w2blk.astype(np.float16)
    w2l = (w2blk - w2h.astype(np.float32)).astype(np.float16)
    b2blk = np.zeros((104, 1), np.float32)
    for si in range(2):
        b2blk[64 * si:64 * si + 40, 0] = np.tile(b2, 4)

    in_maps = []
    for c in range(N_CORES):
        in_maps.append({
            "x": Xc[c], "w1blk": w1blk, "b1blk": b1blk,
            "w2h": w2h, "w2l": w2l, "b2blk": b2blk,
        })
    return in_maps


def kernel(x, W1, b1, W2, b2):
    if "nc" not in _CACHE:
        _CACHE["nc"] = _build_nc()
    nc = _CACHE["nc"]
    in_maps = _host_prep(x, W1, b1, W2, b2)
    res = run_bass_kernel_spmd(nc, in_maps, core_ids=list(range(N_CORES)))
    outs = []
    for c in range(N_CORES):
        y = res.results[c]["y"]          # [10, 8192]
        outs.append(np.ascontiguousarray(y.T))   # [8192, 10]
    return np.concatenate(outs, axis=0).astype(np.float32)
